# revision 32
# baseline (speedup 1.0000x reference)
"""Multi-head attention (B=2, S=2048, D=1024, H=16) on one TRN2 chip (8 cores).

Sharding (Megatron-style): DP=2 over batch x TP=4 over heads.
Core c (c = 0..7): batch g = c//4, heads [4r, 4r+4) where r = c%4.

Per-core pipeline (inputs are host-transposed to x^T [D, S]; all matmuls
bf16 by default):
  - Q^T/K^T [256, S] and V [S, 256] projections (fp32 accum in PSUM).
  - attention per head in "scores transposed" layout (scores^T[k, q]):
    * score matmuls for a head pair run row-tiled (tile_position rows
      0/64) concurrently into a 2-bank PSUM tile; ONE wide Exp per pair.
    * PV matmuls run col-tiled (two heads per ctx PSUM bank).
    * softmax denominators via 4 col-tiled ones-matmuls into one bank.
    * normalize: DVE reciprocal + PE broadcast-matmul + DVE multiply
      (nothing on the GPSIMD queue, which is reserved for collectives).
  - per chunk: ctx^T (bf16) is AllGathered across the TP group; each
    core then projects ALL queries against ITS 256 Wo columns (the
    rank-dependence lives in the per-core Wo input slice, so the
    program stays SPMD-symmetric). No ReduceScatter needed.
Host assembles the per-core output column blocks and adds the bias.

Mask handling (kernel inspects the mask input on the host):
  - canonical causal mask -> fast path: upper-triangle key blocks
    skipped, diagonal blocks get an on-device generated additive mask.
  - all-zeros mask -> dense path, no mask applied.
  - anything else -> generic path: mask^T * sqrt(DH) streamed from DRAM
    and added to every score tile (matches exp(s*scale + m) exactly).
"""

from contextlib import ExitStack

import numpy as np

import concourse.bacc as bacc
import concourse.mybir as mybir
import concourse.tile as tile
from concourse.bass_utils import run_bass_kernel_spmd

F32 = mybir.dt.float32
F32R = mybir.dt.float32r
BF16 = mybir.dt.bfloat16
AF = mybir.ActivationFunctionType

H = 16
D = 1024
B = 2
S = 2048
DH = 64
N_CORES = 8
DP = 2                      # data-parallel groups (over batch)
TP = N_CORES // DP          # tensor-parallel cores per group
HPC = H // TP               # heads per core = 4
DHH = HPC * DH              # 256 features per core
NEG = -1e9

P = 128                     # partitions
FD = 512                    # matmul moving free dim (one PSUM bank fp32)


def _emit(tc, io, mask_mode, s, mm_dtype, with_bias=True):
    with ExitStack() as _stk:
        _emit_inner(_stk, tc, io, mask_mode, s, mm_dtype, with_bias)


def _emit_inner(stk, tc, io, mask_mode, s, mm_dtype, with_bias):
    nc = tc.nc
    NQ = s // FD            # query chunks
    NK = s // P             # key tiles
    ND = D // P             # d-model tiles = 8
    NH2 = HPC // 2          # head pairs = 2
    SPC = FD // P           # seq-tiles per chunk = 4

    MDT = {"f32r": F32R, "bf16": BF16, "f32": F32}[mm_dtype]

    const = stk.enter_context(tc.tile_pool(name="const", bufs=1))
    persist = stk.enter_context(tc.tile_pool(name="persist", bufs=1))
    dram = stk.enter_context(tc.tile_pool(name="dram", bufs=1, space="DRAM"))

    # ---- constants -------------------------------------------------------
    ones_f32 = const.tile([1, FD], F32)
    nc.vector.memset(ones_f32, 1.0)
    ones = const.tile([1, FD], MDT)
    nc.vector.tensor_copy(ones, ones_f32)
    den_ones = const.tile([P, 1], MDT)
    nc.vector.memset(den_ones, 1.0)
    # head-pair selector for the normalize broadcast matmul:
    # bc[m, q] = sum_r sel[r, m] * recip[r, q] -> rows 0-63 get recip row 0,
    # rows 64-127 get recip row 32 (engine writes need 32-aligned partitions;
    # rows 1-31 are zero so the unused recip rows never contribute).
    sel_f = const.tile([33, P], F32)
    nc.vector.memset(sel_f, 0.0)
    nc.vector.memset(sel_f[0:1, 0:64], 1.0)
    nc.vector.memset(sel_f[32:33, 64:P], 1.0)
    sel = const.tile([33, P], F32R)
    nc.vector.tensor_copy(sel, sel_f)
    zrow = const.tile([1, P], MDT)
    nc.vector.memset(zrow, 0.0)
    orow = const.tile([1, FD], MDT)
    nc.vector.memset(orow, 1.0)
    recip_f = persist.tile([97, FD], F32)
    nc.vector.memset(recip_f, 0.0)
    recip_sb = [persist.tile([33, FD], F32R, name=f"recip_{p_}")
                for p_ in range(HPC // 2)]
    for p_ in range(HPC // 2):
        nc.vector.tensor_copy(recip_sb[p_], recip_f[0:33, :])

    if mask_mode == "causal":
        # triangular mask sub-tile: allowed (0) iff qf - kp >= 0 else NEG
        dmask = const.tile([P, 4, P], F32)
        nc.gpsimd.memset(dmask, 0.0)
        for j in range(4):
            nc.gpsimd.affine_select(
                out=dmask[:, j, :],
                in_=dmask[:, j, :],
                compare_op=mybir.AluOpType.is_ge,
                fill=NEG,
                base=0,
                pattern=[[1, P]],
                channel_multiplier=-1,
            )

    # ---- weights / biases -----------------------------------------------
    def load_w(dst, ap):
        if MDT == BF16:
            nc.gpsimd.dma_start(dst, ap)          # SWDGE casts f32 -> bf16
        else:
            nc.sync.dma_start(dst, ap.bitcast(MDT))

    w_sb = {}
    for name in ("wq", "wk", "wv"):
        w_sb[name] = persist.tile([P, ND, DHH], MDT, name=f"w_{name}")
        load_w(w_sb[name], io[name].rearrange("(a p) o -> p a o", p=P))
    # wo: full-D rows, this core's DHH output columns
    wo_sb = persist.tile([P, ND, DHH], MDT)
    load_w(wo_sb, io["wo"].rearrange("(a p) o -> p a o", p=P))

    b_sb = {}
    if with_bias:
        for name in ("bq", "bk", "bv"):
            b_sb[name] = const.tile([1, DHH], MDT, name=f"b_{name}")
            load_w(b_sb[name], io[name])

    # ---- persistent activations: one tile per seq-chunk -----------------
    qT = [persist.tile([P, NH2, FD], MDT, name=f"qT{i}") for i in range(NQ)]
    kT = [persist.tile([P, NH2, FD], MDT, name=f"kT{i}") for i in range(NQ)]
    v_c = [persist.tile([P, SPC, HPC, DH], MDT, name=f"v{i}")
           for i in range(NQ)]
    ctxT = [persist.tile([P, NH2, FD], MDT, name=f"ctxT{i}")
            for i in range(NQ)]

    scale = 1.0 / float(np.sqrt(DH))
    # collective staging.  The first collective op also pays a ~50us
    # one-time comm-init that runs concurrently with early compute, so the
    # first TWO chunks share one AllGather triggered after chunk 1 -- by
    # then the init has finished in the shadow of chunk 0/1 compute.
    merge_first = NQ >= 2
    nmerge = 2 if merge_first else 1
    stage01 = dram.tile([nmerge, P, NH2, FD], MDT, name="ctx_stage01")
    gath01 = dram.tile([TP, nmerge, P, NH2, FD], MDT, name="ctx_gath01")
    ctx_stage = {i: dram.tile([P, NH2, FD], MDT, name=f"ctx_stage_{i}")
                 for i in range(nmerge, NQ)}
    ctx_gath = {i: dram.tile([TP, P, NH2, FD], MDT, name=f"ctx_gath_{i}")
                for i in range(nmerge, NQ)}
    groups = [list(range(g * TP, (g + 1) * TP)) for g in range(DP)]

    with (
        tc.tile_pool(name="xt", bufs=2) as xt_pool,
        tc.tile_pool(name="xth", bufs=1) as xth_pool,
        tc.tile_pool(name="mm_ps", bufs=1, space="PSUM") as mm_ps_pool,
        tc.tile_pool(name="sc_ps", bufs=2, space="PSUM") as sc_ps_pool,
        tc.tile_pool(name="ctx_ps", bufs=3, space="PSUM") as ctx_ps_pool,
        tc.tile_pool(name="pt", bufs=4) as pt_pool,
        tc.tile_pool(name="mload", bufs=3) as mload_pool,
        tc.tile_pool(name="small", bufs=4) as small_pool,
        tc.tile_pool(name="cg_sb", bufs=2) as cg_pool,
        tc.tile_pool(name="out_sb", bufs=3) as out_sb_pool,
    ):
        # hoist all x^T loads ahead of the stream loop: the bf16 cast-DMAs
        # run on the GPSIMD queue, which also issues the collectives -- if
        # emitted inside the loop they stall behind each AllGather.
        xt_all = {}
        if MDT == BF16:
            for sc in range(NQ):
                for tname in ("xq", "xk", "xv"):
                    xt_c = xth_pool.tile([P, ND, FD], MDT,
                                         tag=f"xt_{tname}_{sc}",
                                         name=f"xt_{tname}_{sc}")
                    nc.gpsimd.dma_start(
                        xt_c,
                        io[tname].rearrange("(a p) t -> p a t", p=P)[
                            :, :, sc * FD:(sc + 1) * FD
                        ],
                    )
                    xt_all[(tname, sc)] = xt_c

        def project_chunk(sc):
            for tname, wname, bname, dstT in (
                ("xq", "wq", "bq", qT),
                ("xk", "wk", "bk", kT),
                ("xv", "wv", "bv", None),
            ):
                if MDT == BF16:
                    xt_c = xt_all[(tname, sc)]
                else:
                    xt_c = xt_pool.tile([P, ND, FD], MDT, tag="xt",
                                        name=f"xt_{tname}_{sc}")
                    nc.sync.dma_start(
                        xt_c,
                        io[tname].rearrange("(a p) t -> p a t", p=P)[
                            :, :, sc * FD:(sc + 1) * FD
                        ].bitcast(MDT),
                    )
                if dstT is not None:
                    for mt in range(NH2):
                        qps = mm_ps_pool.tile([P, FD], F32, tag="mm",
                                              name=f"qps_{tname}_{sc}_{mt}")
                        for dt in range(ND):
                            nc.tensor.matmul(
                                qps,
                                w_sb[wname][:, dt, mt * P:(mt + 1) * P],
                                xt_c[:, dt, :],
                                start=(dt == 0),
                                stop=(not with_bias and dt == ND - 1),
                            )
                        if with_bias:
                            nc.tensor.matmul(  # + bias (ones-row augment)
                                qps,
                                b_sb[bname][0:1, mt * P:(mt + 1) * P],
                                ones[0:1, :],
                                start=False,
                                stop=True,
                            )
                        nc.vector.tensor_copy(dstT[sc][:, mt, :], qps)
                else:
                    for st in range(SPC):
                        vp = mm_ps_pool.tile([P, DHH], F32, tag="mm",
                                             name=f"vps_{sc}_{st}")
                        for dt in range(ND):
                            nc.tensor.matmul(
                                vp,
                                xt_c[:, dt, st * P:(st + 1) * P],
                                w_sb[wname][:, dt, :],
                                start=(dt == 0),
                                stop=(not with_bias and dt == ND - 1),
                            )
                        if with_bias:
                            nc.tensor.matmul(
                                vp,
                                ones[0:1, 0:P],
                                b_sb[bname][0:1, :],
                                start=False,
                                stop=True,
                            )
                        nc.vector.tensor_copy(
                            v_c[sc][:, st, :, :],
                            vp.rearrange("p (h e) -> p h e", h=HPC),
                        )

        def attend_chunk(qc):
            nkt = (qc + 1) * SPC if mask_mode == "causal" else NK
            # ctx[p]: two heads of pair p col-tiled into one bank
            # den: 4 heads' softmax denominators at partitions 0/32/64/96
            ctx = [
                ctx_ps_pool.tile([P, FD], F32, tag="ctx",
                                 name=f"ctx_{qc}_{p_}")
                for p_ in range(NH2)
            ]
            den = ctx_ps_pool.tile([P, FD], F32, tag="ctx",
                                   name=f"den_{qc}")
            # one full-bank clearing matmul per accumulator bank (zeros with
            # every has_written bit set) so the col-tiled partial-partition
            # matmuls below can all run start=False -- no ordering hazard
            # between accumulation groups sharing a bank.
            for acc in (ctx[0], ctx[1], den):
                nc.tensor.matmul(acc, zrow, orow, start=True, stop=False,
                                 skip_group_check=True)
            for kt in range(nkt):
                ksc, kti = kt // SPC, kt % SPC
                dj = kt - qc * SPC
                mt_sb = None
                if mask_mode == "generic":
                    mt_sb = mload_pool.tile([P, FD], F32, tag="ml")
                    nc.sync.dma_start(
                        mt_sb,
                        io["maskT"][kt * P:(kt + 1) * P,
                                    qc * FD:(qc + 1) * FD],
                    )
                # causal diagonal tiles: queries below 128*dj see nothing
                # of this key tile -- compute only the valid q-range and
                # mask only the [P, P] sub-tile crossing the diagonal.
                q0 = P * dj if (mask_mode == "causal" and dj > 0) else 0
                for p_ in range(NH2):
                    sp = sc_ps_pool.tile([P, 2, FD], F32, tag="sc",
                                         name=f"sc_{qc}_{kt}_{p_}")
                    for j in range(2):
                        nc.tensor.matmul(
                            sp[:, j, q0:FD],
                            kT[ksc][64 * j:64 * (j + 1), p_,
                                    kti * P:(kti + 1) * P],
                            qT[qc][64 * j:64 * (j + 1), p_, q0:FD],
                            start=True,
                            stop=True,
                        )
                    if mt_sb is not None:
                        for j in range(2):
                            nc.vector.tensor_add(sp[:, j, :], sp[:, j, :],
                                                 mt_sb)
                    elif mask_mode == "causal" and dj >= 0:
                        for j in range(2):
                            nc.vector.tensor_add(
                                sp[:, j, q0:q0 + P], sp[:, j, q0:q0 + P],
                                dmask[:, dj, 0:P],
                            )
                    pt = pt_pool.tile([P, 2, FD], MDT, tag="pt")
                    if q0 == 0:
                        # contiguous 2-bank tile: one flat free dim avoids
                        # the ACT per-row restart overhead
                        nc.scalar.activation(
                            pt.rearrange("p a b -> p (a b)"),
                            sp.rearrange("p a b -> p (a b)"),
                            AF.Exp, scale=scale,
                        )
                    else:
                        nc.scalar.activation(pt[:, :, q0:FD], sp[:, :, q0:FD],
                                             AF.Exp, scale=scale)
                    for j in range(2):
                        hj = 2 * p_ + j
                        nc.tensor.matmul(  # PV, col-tiled pair
                            ctx[p_][64 * j:64 * (j + 1), q0:FD],
                            v_c[ksc][:, kti, hj, :],
                            pt[:, j, q0:FD],
                            start=False,
                            stop=(kt == nkt - 1),
                            skip_group_check=True,
                        )
                        nc.tensor.matmul(  # denominator, col-tiled 4-way
                            den[32 * hj:32 * hj + 1, q0:FD],
                            den_ones,
                            pt[:, j, q0:FD],
                            start=False,
                            stop=(kt == nkt - 1),
                            tile_position=(0, 32 * hj),
                            skip_group_check=True,
                        )
            # normalize: ctxT = ctx * (1/den), broadcast along partitions
            # via a tiny PE matmul (keeps GPSIMD free for collectives).
            # One partition-parallel reciprocal covers all 4 heads (rows
            # 0/32/64/96; the other rows are zeros from the bank clear and
            # their 1/0=inf results are never read).
            nc.vector.reciprocal(recip_f, den[0:97, :])
            # bc tiles live in the score pool (free after the last exp) so
            # the mm pool's FIFO stays clear for the next chunk's projections
            bc_t = sc_ps_pool.tile([P, 2, FD], F32, tag="sc",
                                   name=f"bc_{qc}")
            for p_ in range(NH2):
                recip = recip_sb[p_]
                for j in range(2):
                    hj = 2 * p_ + j
                    nc.vector.tensor_copy(
                        recip[32 * j:32 * j + 1, :],
                        recip_f[32 * hj:32 * hj + 1, :],
                    )
                nc.tensor.matmul(
                    bc_t[:, p_, :],
                    sel,
                    recip,
                    start=True,
                    stop=True,
                )
                bc_sb = small_pool.tile([P, FD], F32, tag="bc_sb",
                                        name=f"bc_sb_{qc}_{p_}")
                nc.vector.tensor_copy(bc_sb, bc_t[:, p_, :])
                nc.vector.tensor_mul(ctxT[qc][:, p_, :], ctx[p_], bc_sb)

        def exchange_chunk(qc):
            if qc < nmerge:
                # merged exchange for chunks [0, nmerge): one AllGather
                for i in range(nmerge):
                    nc.sync.dma_start(stage01[i], ctxT[i])
                nc.gpsimd.collective_compute(
                    "AllGather",
                    mybir.AluOpType.bypass,
                    replica_groups=groups,
                    ins=[stage01.opt()],
                    outs=[gath01.opt()],
                )
            else:
                nc.sync.dma_start(ctx_stage[qc], ctxT[qc])
                nc.gpsimd.collective_compute(
                    "AllGather",
                    mybir.AluOpType.bypass,
                    replica_groups=groups,
                    ins=[ctx_stage[qc].opt()],
                    outs=[ctx_gath[qc].opt()],
                )

        def project_out_chunk(qc):
            # every core projects ALL queries of this chunk against its
            # own 256 Wo columns (rank-dependence is in the Wo input).
            if qc < nmerge:
                src = gath01[:, qc].rearrange("r p h q -> p r h q")
            else:
                src = ctx_gath[qc].rearrange("r p h q -> p r h q")
            cg = cg_pool.tile([P, TP, NH2, FD], MDT, tag="cg",
                              name=f"cg_{qc}")
            nc.sync.dma_start(cg, src)
            for st in range(SPC):
                op = mm_ps_pool.tile([P, DHH], F32, tag="mm",
                                     name=f"op_{qc}_{st}")
                for r in range(TP):
                    for hp in range(NH2):
                        nc.tensor.matmul(
                            op,
                            cg[:, r, hp, st * P:(st + 1) * P],
                            wo_sb[:, 2 * r + hp, :],
                            start=(r == 0 and hp == 0),
                            stop=(r == TP - 1 and hp == NH2 - 1),
                        )
                ob = out_sb_pool.tile([P, DHH], F32, tag="ob")
                nc.vector.tensor_copy(ob, op)
                nc.sync.dma_start(
                    io["out"][qc, st * P:(st + 1) * P, :], ob
                )

        # stream: chunk qc's attention needs only K/V chunks <= qc (causal),
        # so interleave projection and attention per chunk.  The output
        # projection of chunk qc is emitted two chunks later: its PSUM
        # tiles wait on the AllGather, and emitting them late keeps that
        # wait from serializing the (FIFO) psum pool rotation.
        def emit_exchange(sc):
            if merge_first:
                if sc == 1:
                    exchange_chunk(0)
                elif sc >= 2:
                    exchange_chunk(sc)
            else:
                exchange_chunk(sc)

        if mask_mode == "causal":
            for sc in range(NQ):
                project_chunk(sc)
                attend_chunk(sc)
                emit_exchange(sc)
                if sc >= 2:
                    project_out_chunk(sc - 2)
        else:
            for sc in range(NQ):
                project_chunk(sc)
            for qc in range(NQ):
                attend_chunk(qc)
                emit_exchange(qc)
                if qc >= 2:
                    project_out_chunk(qc - 2)
        for qc in range(max(0, NQ - 2), NQ):
            project_out_chunk(qc)


def build(mask_mode="causal", s=S, mm_dtype="f32r", with_bias=True):
    """Build the SPMD Bass module for one core."""
    assert mask_mode in ("causal", "zeros", "generic")
    assert mm_dtype in ("f32r", "bf16", "f32")
    assert s % FD == 0
    nc = bacc.Bacc(
        "TRN2", target_bir_lowering=False, debug=False, num_devices=N_CORES
    )
    io = {}
    for name in ("xq", "xk", "xv"):
        # host passes x^T: [D, s]
        io[name] = nc.dram_tensor(name, [D, s], F32, kind="ExternalInput").ap()
    for name in ("wq", "wk", "wv"):
        io[name] = nc.dram_tensor(name, [D, DHH], F32, kind="ExternalInput").ap()
    # wo: full rows, this core's DHH columns
    io["wo"] = nc.dram_tensor("wo", [D, DHH], F32, kind="ExternalInput").ap()
    for name in ("bq", "bk", "bv"):
        io[name] = nc.dram_tensor(name, [1, DHH], F32, kind="ExternalInput").ap()
    if mask_mode == "generic":
        io["maskT"] = nc.dram_tensor(
            "maskT", [s, s], F32, kind="ExternalInput"
        ).ap()
    # output: all queries, this core's DHH output columns
    io["out"] = nc.dram_tensor(
        "out", [s // FD, FD, DHH], F32, kind="ExternalOutput"
    ).ap()

    with tile.TileContext(nc) as tc:
        _emit(tc, io, mask_mode, s, mm_dtype, with_bias)
    nc.compile()
    return nc


def detect_mask_mode(mask, s=S):
    m = np.asarray(mask).reshape(s, s)
    if not np.any(m):
        return "zeros"
    causal = np.where(
        np.tril(np.ones((s, s), dtype=bool)), 0.0, np.float32(NEG)
    ).astype(np.float32)
    if np.array_equal(m, causal):
        return "causal"
    return "generic"


def make_in_maps(q, k, v, mask, Wq, bq, Wk, bk, Wv, bv, Wo, bo, mask_mode,
                 s=S):
    c32 = lambda a: np.ascontiguousarray(a, dtype=np.float32)
    # one host-side transpose per (batch, tensor), shared by the TP group
    xT = [[c32(np.asarray(t)[g].T) for t in (q, k, v)] for g in range(DP)]
    in_maps = []
    for c in range(N_CORES):
        g, r = c // TP, c % TP
        sl = slice(r * DHH, (r + 1) * DHH)
        m = {
            "xq": xT[g][0], "xk": xT[g][1], "xv": xT[g][2],
            "wq": c32(Wq[:, sl]), "wk": c32(Wk[:, sl]), "wv": c32(Wv[:, sl]),
            "wo": c32(Wo[:, sl]),
            "bq": c32(bq[sl]).reshape(1, DHH),
            "bk": c32(bk[sl]).reshape(1, DHH),
            "bv": c32(bv[sl]).reshape(1, DHH),
        }
        if mask_mode == "generic":
            # pre-scaled by sqrt(DH) so exp((s + m*8)/8) == exp(s/8 + m)
            m["maskT"] = c32(
                np.asarray(mask).reshape(s, s).T * np.float32(DH) ** 0.5
            )
        in_maps.append(m)
    return in_maps


def assemble(results, bo, s=S):
    out = np.empty((B, s, D), np.float32)
    for c in range(N_CORES):
        g, r = c // TP, c % TP
        piece = np.asarray(results[c]["out"]).reshape(s, DHH)
        out[g, :, r * DHH:(r + 1) * DHH] = piece
    out += np.asarray(bo, dtype=np.float32)[None, None, :]
    return out


_cache = {}
MM_DTYPE = "bf16"


def kernel(q, k, v, mask, Wq, bq, Wk, bk, Wv, bv, Wo, bo):
    mask_mode = detect_mask_mode(mask)
    with_bias = any(np.any(np.asarray(b)) for b in (bq, bk, bv))
    key = (mask_mode, with_bias)
    if key not in _cache:
        _cache[key] = build(mask_mode=mask_mode, mm_dtype=MM_DTYPE,
                            with_bias=with_bias)
    nc = _cache[key]
    in_maps = make_in_maps(
        q, k, v, mask, Wq, bq, Wk, bk, Wv, bv, Wo, bo, mask_mode
    )
    res = run_bass_kernel_spmd(nc, in_maps, list(range(N_CORES)))
    return assemble(res.results, bo)


# revision 33
# speedup vs baseline: 1.2947x; 1.2947x over previous
"""Multi-head attention (B=2, S=2048, D=1024, H=16) on one TRN2 chip (8 cores).

Sharding (Megatron-style): DP=2 over batch x TP=4 over heads.
Core c (c = 0..7): batch g = c//4, heads [4r, 4r+4) where r = c%4.

Per-core pipeline (inputs are host-transposed to x^T [D, S]; all matmuls
bf16 by default):
  - Q^T/K^T [256, S] and V [S, 256] projections (fp32 accum in PSUM).
  - attention per head in "scores transposed" layout (scores^T[k, q]):
    * score matmuls for a head pair run row-tiled (tile_position rows
      0/64) concurrently into a 2-bank PSUM tile; ONE wide Exp per pair.
    * PV matmuls run col-tiled (two heads per ctx PSUM bank).
    * softmax denominators via 4 col-tiled ones-matmuls into one bank.
    * normalize: DVE reciprocal + PE broadcast-matmul + DVE multiply
      (nothing on the GPSIMD queue, which is reserved for collectives).
  - per chunk: ctx^T (bf16) is AllGathered across the TP group; each
    core then projects ALL queries against ITS 256 Wo columns (the
    rank-dependence lives in the per-core Wo input slice, so the
    program stays SPMD-symmetric). No ReduceScatter needed.
Host assembles the per-core output column blocks and adds the bias.

Mask handling (kernel inspects the mask input on the host):
  - canonical causal mask -> fast path: upper-triangle key blocks
    skipped, diagonal blocks get an on-device generated additive mask.
  - all-zeros mask -> dense path, no mask applied.
  - anything else -> generic path: mask^T * sqrt(DH) streamed from DRAM
    and added to every score tile (matches exp(s*scale + m) exactly).
"""

from contextlib import ExitStack

import numpy as np

import concourse.bacc as bacc
import concourse.mybir as mybir
import concourse.tile as tile
from concourse.bass_utils import run_bass_kernel_spmd

F32 = mybir.dt.float32
F32R = mybir.dt.float32r
BF16 = mybir.dt.bfloat16
AF = mybir.ActivationFunctionType

H = 16
D = 1024
B = 2
S = 2048
DH = 64
N_CORES = 8
DP = 2                      # data-parallel groups (over batch)
TP = N_CORES // DP          # tensor-parallel cores per group
HPC = H // TP               # heads per core = 4
DHH = HPC * DH              # 256 features per core
NEG = -1e9

P = 128                     # partitions
FD = 512                    # matmul moving free dim (one PSUM bank fp32)


def _emit(tc, io, mask_mode, s, mm_dtype, with_bias=True):
    with ExitStack() as _stk:
        _emit_inner(_stk, tc, io, mask_mode, s, mm_dtype, with_bias)


def _emit_inner(stk, tc, io, mask_mode, s, mm_dtype, with_bias):
    nc = tc.nc
    NQ = s // FD            # query chunks
    NK = s // P             # key tiles
    ND = D // P             # d-model tiles = 8
    NH2 = HPC // 2          # head pairs = 2
    SPC = FD // P           # seq-tiles per chunk = 4

    MDT = {"f32r": F32R, "bf16": BF16, "f32": F32}[mm_dtype]

    const = stk.enter_context(tc.tile_pool(name="const", bufs=1))
    persist = stk.enter_context(tc.tile_pool(name="persist", bufs=1))
    dram = stk.enter_context(tc.tile_pool(name="dram", bufs=1, space="DRAM"))

    # ---- constants -------------------------------------------------------
    ones_f32 = const.tile([1, FD], F32)
    nc.vector.memset(ones_f32, 1.0)
    ones = const.tile([1, FD], MDT)
    nc.vector.tensor_copy(ones, ones_f32)
    den_ones = const.tile([P, 1], MDT)
    nc.vector.memset(den_ones, 1.0)
    # head-pair selector for the normalize broadcast matmul:
    # bc[m, q] = sum_r sel[r, m] * recip[r, q] -> rows 0-63 get recip row 0,
    # rows 64-127 get recip row 32 (engine writes need 32-aligned partitions;
    # rows 1-31 are zero so the unused recip rows never contribute).
    sel_f = const.tile([33, P], F32)
    nc.vector.memset(sel_f, 0.0)
    nc.vector.memset(sel_f[0:1, 0:64], 1.0)
    nc.vector.memset(sel_f[32:33, 64:P], 1.0)
    sel = const.tile([33, P], F32R)
    nc.vector.tensor_copy(sel, sel_f)
    zrow = const.tile([1, P], MDT)
    nc.vector.memset(zrow, 0.0)
    orow = const.tile([1, FD], MDT)
    nc.vector.memset(orow, 1.0)
    recip_f = persist.tile([97, FD], F32)
    nc.vector.memset(recip_f, 0.0)
    recip_sb = [persist.tile([33, FD], F32R, name=f"recip_{p_}")
                for p_ in range(HPC // 2)]
    for p_ in range(HPC // 2):
        nc.vector.tensor_copy(recip_sb[p_], recip_f[0:33, :])

    if mask_mode == "causal":
        # triangular mask sub-tile: allowed (0) iff qf - kp >= 0 else NEG
        dmask = const.tile([P, 4, P], F32)
        nc.gpsimd.memset(dmask, 0.0)
        for j in range(4):
            nc.gpsimd.affine_select(
                out=dmask[:, j, :],
                in_=dmask[:, j, :],
                compare_op=mybir.AluOpType.is_ge,
                fill=NEG,
                base=0,
                pattern=[[1, P]],
                channel_multiplier=-1,
            )

    # ---- weights / biases -----------------------------------------------
    def load_w(dst, ap):
        if MDT == BF16:
            nc.gpsimd.dma_start(dst, ap)          # SWDGE casts f32 -> bf16
        else:
            nc.sync.dma_start(dst, ap.bitcast(MDT))

    w_sb = {}
    for name in ("wq", "wk", "wv"):
        w_sb[name] = persist.tile([P, ND, DHH], MDT, name=f"w_{name}")
        load_w(w_sb[name], io[name].rearrange("(a p) o -> p a o", p=P))
    # wo: full-D rows, this core's DHH output columns
    wo_sb = persist.tile([P, ND, DHH], MDT)
    load_w(wo_sb, io["wo"].rearrange("(a p) o -> p a o", p=P))

    b_sb = {}
    if with_bias:
        for name in ("bq", "bk", "bv"):
            b_sb[name] = const.tile([1, DHH], MDT, name=f"b_{name}")
            load_w(b_sb[name], io[name])

    # ---- persistent activations: one tile per seq-chunk -----------------
    qT = [persist.tile([P, NH2, FD], MDT, name=f"qT{i}") for i in range(NQ)]
    kT = [persist.tile([P, NH2, FD], MDT, name=f"kT{i}") for i in range(NQ)]
    v_c = [persist.tile([P, SPC, HPC, DH], MDT, name=f"v{i}")
           for i in range(NQ)]
    ctxT = [persist.tile([P, NH2, FD], MDT, name=f"ctxT{i}")
            for i in range(NQ)]

    scale = 1.0 / float(np.sqrt(DH))
    # collective staging.  The first collective op also pays a ~50us
    # one-time comm-init that runs concurrently with early compute, so the
    # first TWO chunks share one AllGather triggered after chunk 1 -- by
    # then the init has finished in the shadow of chunk 0/1 compute.
    merge_first = NQ >= 2
    nmerge = 2 if merge_first else 1
    stage01 = dram.tile([nmerge, P, NH2, FD], MDT, name="ctx_stage01")
    gath01 = dram.tile([TP, nmerge, P, NH2, FD], MDT, name="ctx_gath01")
    ctx_stage = {i: dram.tile([P, NH2, FD], MDT, name=f"ctx_stage_{i}")
                 for i in range(nmerge, NQ)}
    ctx_gath = {i: dram.tile([TP, P, NH2, FD], MDT, name=f"ctx_gath_{i}")
                for i in range(nmerge, NQ)}
    groups = [list(range(g * TP, (g + 1) * TP)) for g in range(DP)]

    with (
        tc.tile_pool(name="xt", bufs=2) as xt_pool,
        tc.tile_pool(name="xth", bufs=1) as xth_pool,
        tc.tile_pool(name="mm_ps", bufs=1, space="PSUM") as mm_ps_pool,
        tc.tile_pool(name="sc_ps", bufs=2, space="PSUM") as sc_ps_pool,
        tc.tile_pool(name="ctx_ps", bufs=3, space="PSUM") as ctx_ps_pool,
        tc.tile_pool(name="pt", bufs=4) as pt_pool,
        tc.tile_pool(name="mload", bufs=3) as mload_pool,
        tc.tile_pool(name="small", bufs=4) as small_pool,
        tc.tile_pool(name="cg_sb", bufs=2) as cg_pool,
        tc.tile_pool(name="out_sb", bufs=3) as out_sb_pool,
    ):
        # hoist all x^T loads ahead of the stream loop: the bf16 cast-DMAs
        # run on the GPSIMD queue, which also issues the collectives -- if
        # emitted inside the loop they stall behind each AllGather.
        xt_all = {}
        if MDT == BF16:
            for sc in range(NQ):
                for tname in ("xq", "xk", "xv"):
                    xt_c = xth_pool.tile([P, ND, FD], MDT,
                                         tag=f"xt_{tname}_{sc}",
                                         name=f"xt_{tname}_{sc}")
                    nc.gpsimd.dma_start(
                        xt_c,
                        io[tname].rearrange("(a p) t -> p a t", p=P)[
                            :, :, sc * FD:(sc + 1) * FD
                        ],
                    )
                    xt_all[(tname, sc)] = xt_c

        def project_chunk(sc):
            for tname, wname, bname, dstT in (
                ("xq", "wq", "bq", qT),
                ("xk", "wk", "bk", kT),
                ("xv", "wv", "bv", None),
            ):
                if MDT == BF16:
                    xt_c = xt_all[(tname, sc)]
                else:
                    xt_c = xt_pool.tile([P, ND, FD], MDT, tag="xt",
                                        name=f"xt_{tname}_{sc}")
                    nc.sync.dma_start(
                        xt_c,
                        io[tname].rearrange("(a p) t -> p a t", p=P)[
                            :, :, sc * FD:(sc + 1) * FD
                        ].bitcast(MDT),
                    )
                if dstT is not None:
                    for mt in range(NH2):
                        qps = mm_ps_pool.tile([P, FD], F32, tag="mm",
                                              name=f"qps_{tname}_{sc}_{mt}")
                        for dt in range(ND):
                            nc.tensor.matmul(
                                qps,
                                w_sb[wname][:, dt, mt * P:(mt + 1) * P],
                                xt_c[:, dt, :],
                                start=(dt == 0),
                                stop=(not with_bias and dt == ND - 1),
                            )
                        if with_bias:
                            nc.tensor.matmul(  # + bias (ones-row augment)
                                qps,
                                b_sb[bname][0:1, mt * P:(mt + 1) * P],
                                ones[0:1, :],
                                start=False,
                                stop=True,
                            )
                        nc.vector.tensor_copy(dstT[sc][:, mt, :], qps)
                else:
                    for st in range(SPC):
                        vp = mm_ps_pool.tile([P, DHH], F32, tag="mm",
                                             name=f"vps_{sc}_{st}")
                        for dt in range(ND):
                            nc.tensor.matmul(
                                vp,
                                xt_c[:, dt, st * P:(st + 1) * P],
                                w_sb[wname][:, dt, :],
                                start=(dt == 0),
                                stop=(not with_bias and dt == ND - 1),
                            )
                        if with_bias:
                            nc.tensor.matmul(
                                vp,
                                ones[0:1, 0:P],
                                b_sb[bname][0:1, :],
                                start=False,
                                stop=True,
                            )
                        nc.vector.tensor_copy(
                            v_c[sc][:, st, :, :],
                            vp.rearrange("p (h e) -> p h e", h=HPC),
                        )

        def attend_chunk(qc):
            nkt = (qc + 1) * SPC if mask_mode == "causal" else NK
            # ctx[p]: two heads of pair p col-tiled into one bank
            # den: 4 heads' softmax denominators at partitions 0/32/64/96
            ctx = [
                ctx_ps_pool.tile([P, FD], F32, tag="ctx",
                                 name=f"ctx_{qc}_{p_}")
                for p_ in range(NH2)
            ]
            den = ctx_ps_pool.tile([P, FD], F32, tag="ctx",
                                   name=f"den_{qc}")
            # one full-bank clearing matmul per accumulator bank (zeros with
            # every has_written bit set) so the col-tiled partial-partition
            # matmuls below can all run start=False -- no ordering hazard
            # between accumulation groups sharing a bank.
            for acc in (ctx[0], ctx[1], den):
                nc.tensor.matmul(acc, zrow, orow, start=True, stop=False,
                                 skip_group_check=True)
            for kt in range(nkt):
                ksc, kti = kt // SPC, kt % SPC
                dj = kt - qc * SPC
                mt_sb = None
                if mask_mode == "generic":
                    mt_sb = mload_pool.tile([P, FD], F32, tag="ml")
                    nc.sync.dma_start(
                        mt_sb,
                        io["maskT"][kt * P:(kt + 1) * P,
                                    qc * FD:(qc + 1) * FD],
                    )
                # causal diagonal tiles: queries below 128*dj see nothing
                # of this key tile -- compute only the valid q-range and
                # mask only the [P, P] sub-tile crossing the diagonal.
                q0 = P * dj if (mask_mode == "causal" and dj > 0) else 0
                for p_ in range(NH2):
                    sp = sc_ps_pool.tile([P, 2, FD], F32, tag="sc",
                                         name=f"sc_{qc}_{kt}_{p_}")
                    for j in range(2):
                        nc.tensor.matmul(
                            sp[:, j, q0:FD],
                            kT[ksc][64 * j:64 * (j + 1), p_,
                                    kti * P:(kti + 1) * P],
                            qT[qc][64 * j:64 * (j + 1), p_, q0:FD],
                            start=True,
                            stop=True,
                        )
                    if mt_sb is not None:
                        for j in range(2):
                            nc.vector.tensor_add(sp[:, j, :], sp[:, j, :],
                                                 mt_sb)
                    elif mask_mode == "causal" and dj >= 0:
                        for j in range(2):
                            nc.vector.tensor_add(
                                sp[:, j, q0:q0 + P], sp[:, j, q0:q0 + P],
                                dmask[:, dj, 0:P],
                            )
                    pt = pt_pool.tile([P, 2, FD], MDT, tag="pt")
                    if q0 == 0:
                        # contiguous 2-bank tile: one flat free dim avoids
                        # the ACT per-row restart overhead
                        nc.scalar.activation(
                            pt.rearrange("p a b -> p (a b)"),
                            sp.rearrange("p a b -> p (a b)"),
                            AF.Exp, scale=scale,
                        )
                    else:
                        nc.scalar.activation(pt[:, :, q0:FD], sp[:, :, q0:FD],
                                             AF.Exp, scale=scale)
                    for j in range(2):
                        hj = 2 * p_ + j
                        nc.tensor.matmul(  # PV, col-tiled pair
                            ctx[p_][64 * j:64 * (j + 1), q0:FD],
                            v_c[ksc][:, kti, hj, :],
                            pt[:, j, q0:FD],
                            start=False,
                            stop=(kt == nkt - 1),
                            skip_group_check=True,
                        )
                        nc.tensor.matmul(  # denominator, col-tiled 4-way
                            den[32 * hj:32 * hj + 1, q0:FD],
                            den_ones,
                            pt[:, j, q0:FD],
                            start=False,
                            stop=(kt == nkt - 1),
                            tile_position=(0, 32 * hj),
                            skip_group_check=True,
                        )
            # normalize: ctxT = ctx * (1/den), broadcast along partitions
            # via a tiny PE matmul (keeps GPSIMD free for collectives).
            # One partition-parallel reciprocal covers all 4 heads (rows
            # 0/32/64/96; the other rows are zeros from the bank clear and
            # their 1/0=inf results are never read).
            nc.vector.reciprocal(recip_f, den[0:97, :])
            # bc tiles live in the score pool (free after the last exp) so
            # the mm pool's FIFO stays clear for the next chunk's projections
            bc_t = sc_ps_pool.tile([P, 2, FD], F32, tag="sc",
                                   name=f"bc_{qc}")
            for p_ in range(NH2):
                recip = recip_sb[p_]
                for j in range(2):
                    hj = 2 * p_ + j
                    nc.vector.tensor_copy(
                        recip[32 * j:32 * j + 1, :],
                        recip_f[32 * hj:32 * hj + 1, :],
                    )
                nc.tensor.matmul(
                    bc_t[:, p_, :],
                    sel,
                    recip,
                    start=True,
                    stop=True,
                )
                bc_sb = small_pool.tile([P, FD], F32, tag="bc_sb",
                                        name=f"bc_sb_{qc}_{p_}")
                nc.vector.tensor_copy(bc_sb, bc_t[:, p_, :])
                nc.vector.tensor_mul(ctxT[qc][:, p_, :], ctx[p_], bc_sb)

        def exchange_chunk(qc):
            if qc < nmerge:
                # merged exchange for chunks [0, nmerge): one AllGather
                for i in range(nmerge):
                    nc.sync.dma_start(stage01[i], ctxT[i])
                nc.gpsimd.collective_compute(
                    "AllGather",
                    mybir.AluOpType.bypass,
                    replica_groups=groups,
                    ins=[stage01.opt()],
                    outs=[gath01.opt()],
                )
            else:
                nc.sync.dma_start(ctx_stage[qc], ctxT[qc])
                nc.gpsimd.collective_compute(
                    "AllGather",
                    mybir.AluOpType.bypass,
                    replica_groups=groups,
                    ins=[ctx_stage[qc].opt()],
                    outs=[ctx_gath[qc].opt()],
                )

        def project_out_chunk(qc):
            # every core projects ALL queries of this chunk against its
            # own 256 Wo columns (rank-dependence is in the Wo input).
            if qc < nmerge:
                src = gath01[:, qc].rearrange("r p h q -> p r h q")
            else:
                src = ctx_gath[qc].rearrange("r p h q -> p r h q")
            cg = cg_pool.tile([P, TP, NH2, FD], MDT, tag="cg",
                              name=f"cg_{qc}")
            nc.sync.dma_start(cg, src)
            for st in range(SPC):
                op = mm_ps_pool.tile([P, DHH], F32, tag="mm",
                                     name=f"op_{qc}_{st}")
                for r in range(TP):
                    for hp in range(NH2):
                        nc.tensor.matmul(
                            op,
                            cg[:, r, hp, st * P:(st + 1) * P],
                            wo_sb[:, 2 * r + hp, :],
                            start=(r == 0 and hp == 0),
                            stop=(r == TP - 1 and hp == NH2 - 1),
                        )
                ob = out_sb_pool.tile([P, DHH], F32, tag="ob")
                nc.vector.tensor_copy(ob, op)
                nc.sync.dma_start(
                    io["out"][qc, st * P:(st + 1) * P, :], ob
                )

        # stream: chunk qc's attention needs only K/V chunks <= qc (causal),
        # so interleave projection and attention per chunk.  The output
        # projection of chunk qc is emitted two chunks later: its PSUM
        # tiles wait on the AllGather, and emitting them late keeps that
        # wait from serializing the (FIFO) psum pool rotation.
        def emit_exchange(sc):
            if merge_first:
                if sc == 1:
                    exchange_chunk(0)
                elif sc >= 2:
                    exchange_chunk(sc)
            else:
                exchange_chunk(sc)

        if mask_mode == "causal":
            for sc in range(NQ):
                project_chunk(sc)
                attend_chunk(sc)
                emit_exchange(sc)
        else:
            for sc in range(NQ):
                project_chunk(sc)
            for qc in range(NQ):
                attend_chunk(qc)
                emit_exchange(qc)
        # all output projections AFTER the stream: their PSUM tiles queue
        # behind the last projections in the (FIFO) mm pool, so they fill
        # the last chunk's exp-bound PE gaps instead of blocking the stream
        # on AllGather completion.
        for qc in range(NQ):
            project_out_chunk(qc)


def build(mask_mode="causal", s=S, mm_dtype="f32r", with_bias=True):
    """Build the SPMD Bass module for one core."""
    assert mask_mode in ("causal", "zeros", "generic")
    assert mm_dtype in ("f32r", "bf16", "f32")
    assert s % FD == 0
    nc = bacc.Bacc(
        "TRN2", target_bir_lowering=False, debug=False, num_devices=N_CORES
    )
    io = {}
    for name in ("xq", "xk", "xv"):
        # host passes x^T: [D, s]
        io[name] = nc.dram_tensor(name, [D, s], F32, kind="ExternalInput").ap()
    for name in ("wq", "wk", "wv"):
        io[name] = nc.dram_tensor(name, [D, DHH], F32, kind="ExternalInput").ap()
    # wo: full rows, this core's DHH columns
    io["wo"] = nc.dram_tensor("wo", [D, DHH], F32, kind="ExternalInput").ap()
    for name in ("bq", "bk", "bv"):
        io[name] = nc.dram_tensor(name, [1, DHH], F32, kind="ExternalInput").ap()
    if mask_mode == "generic":
        io["maskT"] = nc.dram_tensor(
            "maskT", [s, s], F32, kind="ExternalInput"
        ).ap()
    # output: all queries, this core's DHH output columns
    io["out"] = nc.dram_tensor(
        "out", [s // FD, FD, DHH], F32, kind="ExternalOutput"
    ).ap()

    with tile.TileContext(nc) as tc:
        _emit(tc, io, mask_mode, s, mm_dtype, with_bias)
    nc.compile()
    return nc


def detect_mask_mode(mask, s=S):
    m = np.asarray(mask).reshape(s, s)
    if not np.any(m):
        return "zeros"
    causal = np.where(
        np.tril(np.ones((s, s), dtype=bool)), 0.0, np.float32(NEG)
    ).astype(np.float32)
    if np.array_equal(m, causal):
        return "causal"
    return "generic"


def make_in_maps(q, k, v, mask, Wq, bq, Wk, bk, Wv, bv, Wo, bo, mask_mode,
                 s=S):
    c32 = lambda a: np.ascontiguousarray(a, dtype=np.float32)
    # one host-side transpose per (batch, tensor), shared by the TP group
    xT = [[c32(np.asarray(t)[g].T) for t in (q, k, v)] for g in range(DP)]
    in_maps = []
    for c in range(N_CORES):
        g, r = c // TP, c % TP
        sl = slice(r * DHH, (r + 1) * DHH)
        m = {
            "xq": xT[g][0], "xk": xT[g][1], "xv": xT[g][2],
            "wq": c32(Wq[:, sl]), "wk": c32(Wk[:, sl]), "wv": c32(Wv[:, sl]),
            "wo": c32(Wo[:, sl]),
            "bq": c32(bq[sl]).reshape(1, DHH),
            "bk": c32(bk[sl]).reshape(1, DHH),
            "bv": c32(bv[sl]).reshape(1, DHH),
        }
        if mask_mode == "generic":
            # pre-scaled by sqrt(DH) so exp((s + m*8)/8) == exp(s/8 + m)
            m["maskT"] = c32(
                np.asarray(mask).reshape(s, s).T * np.float32(DH) ** 0.5
            )
        in_maps.append(m)
    return in_maps


def assemble(results, bo, s=S):
    out = np.empty((B, s, D), np.float32)
    for c in range(N_CORES):
        g, r = c // TP, c % TP
        piece = np.asarray(results[c]["out"]).reshape(s, DHH)
        out[g, :, r * DHH:(r + 1) * DHH] = piece
    out += np.asarray(bo, dtype=np.float32)[None, None, :]
    return out


_cache = {}
MM_DTYPE = "bf16"


def kernel(q, k, v, mask, Wq, bq, Wk, bk, Wv, bv, Wo, bo):
    mask_mode = detect_mask_mode(mask)
    with_bias = any(np.any(np.asarray(b)) for b in (bq, bk, bv))
    key = (mask_mode, with_bias)
    if key not in _cache:
        _cache[key] = build(mask_mode=mask_mode, mm_dtype=MM_DTYPE,
                            with_bias=with_bias)
    nc = _cache[key]
    in_maps = make_in_maps(
        q, k, v, mask, Wq, bq, Wk, bk, Wv, bv, Wo, bo, mask_mode
    )
    res = run_bass_kernel_spmd(nc, in_maps, list(range(N_CORES)))
    return assemble(res.results, bo)


# revision 42
# speedup vs baseline: 1.3534x; 1.0453x over previous
"""Multi-head attention (B=2, S=2048, D=1024, H=16) on one TRN2 chip (8 cores).

Sharding (Megatron-style): DP=2 over batch x TP=4 over heads.
Core c (c = 0..7): batch g = c//4, heads [4r, 4r+4) where r = c%4.

Per-core pipeline (inputs are host-transposed to x^T [D, S]; all matmuls
bf16 by default):
  - Q^T/K^T [256, S] and V [S, 256] projections (fp32 accum in PSUM).
  - attention per head in "scores transposed" layout (scores^T[k, q]):
    * score matmuls for a head pair run row-tiled (tile_position rows
      0/64) concurrently into a 2-bank PSUM tile; ONE wide Exp per pair.
    * PV matmuls run col-tiled (two heads per ctx PSUM bank).
    * softmax denominators via 4 col-tiled ones-matmuls into one bank.
    * normalize: DVE reciprocal + PE broadcast-matmul + DVE multiply
      (nothing on the GPSIMD queue, which is reserved for collectives).
  - per chunk: ctx^T (bf16) is AllGathered across the TP group; each
    core then projects ALL queries against ITS 256 Wo columns (the
    rank-dependence lives in the per-core Wo input slice, so the
    program stays SPMD-symmetric). No ReduceScatter needed.
Host assembles the per-core output column blocks and adds the bias.

Mask handling (kernel inspects the mask input on the host):
  - canonical causal mask -> fast path: upper-triangle key blocks
    skipped, diagonal blocks get an on-device generated additive mask.
  - all-zeros mask -> dense path, no mask applied.
  - anything else -> generic path: mask^T * sqrt(DH) streamed from DRAM
    and added to every score tile (matches exp(s*scale + m) exactly).
"""

from contextlib import ExitStack

import numpy as np

import concourse.bacc as bacc
import concourse.mybir as mybir
import concourse.tile as tile
from concourse.bass_utils import run_bass_kernel_spmd

F32 = mybir.dt.float32
F32R = mybir.dt.float32r
BF16 = mybir.dt.bfloat16
AF = mybir.ActivationFunctionType

H = 16
D = 1024
B = 2
S = 2048
DH = 64
N_CORES = 8
DP = 2                      # data-parallel groups (over batch)
TP = N_CORES // DP          # tensor-parallel cores per group
HPC = H // TP               # heads per core = 4
DHH = HPC * DH              # 256 features per core
NEG = -1e9

P = 128                     # partitions
FD = 512                    # matmul moving free dim (one PSUM bank fp32)


def _emit(tc, io, mask_mode, s, mm_dtype, with_bias=True):
    with ExitStack() as _stk:
        _emit_inner(_stk, tc, io, mask_mode, s, mm_dtype, with_bias)


def _emit_inner(stk, tc, io, mask_mode, s, mm_dtype, with_bias):
    nc = tc.nc
    NQ = s // FD            # query chunks
    NK = s // P             # key tiles
    ND = D // P             # d-model tiles = 8
    NH2 = HPC // 2          # head pairs = 2
    SPC = FD // P           # seq-tiles per chunk = 4

    MDT = {"f32r": F32R, "bf16": BF16, "f32": F32}[mm_dtype]

    const = stk.enter_context(tc.tile_pool(name="const", bufs=1))
    persist = stk.enter_context(tc.tile_pool(name="persist", bufs=1))
    dram = stk.enter_context(tc.tile_pool(name="dram", bufs=1, space="DRAM"))

    # ---- constants -------------------------------------------------------
    ones_f32 = const.tile([1, FD], F32)
    nc.vector.memset(ones_f32, 1.0)
    ones = const.tile([1, FD], MDT)
    nc.vector.tensor_copy(ones, ones_f32)
    den_ones = const.tile([P, 1], MDT)
    nc.vector.memset(den_ones, 1.0)
    # head-pair selector for the normalize broadcast matmul:
    # bc[m, q] = sum_r sel[r, m] * recip[r, q] -> rows 0-63 get recip row 0,
    # rows 64-127 get recip row 32 (engine writes need 32-aligned partitions;
    # rows 1-31 are zero so the unused recip rows never contribute).
    sel_f = const.tile([33, P], F32)
    nc.vector.memset(sel_f, 0.0)
    nc.vector.memset(sel_f[0:1, 0:64], 1.0)
    nc.vector.memset(sel_f[32:33, 64:P], 1.0)
    sel = const.tile([33, P], F32R)
    nc.vector.tensor_copy(sel, sel_f)
    zrow = const.tile([1, P], MDT)
    nc.vector.memset(zrow, 0.0)
    orow = const.tile([1, FD], MDT)
    nc.vector.memset(orow, 1.0)
    recip_f = persist.tile([97, FD], F32)
    nc.vector.memset(recip_f, 0.0)
    recip_sb = [persist.tile([33, FD], F32R, name=f"recip_{p_}")
                for p_ in range(HPC // 2)]
    for p_ in range(HPC // 2):
        nc.vector.tensor_copy(recip_sb[p_], recip_f[0:33, :])

    if mask_mode == "causal":
        # triangular mask sub-tile: allowed (0) iff qf - kp >= 0 else NEG
        dmask = const.tile([P, 4, P], F32)
        nc.gpsimd.memset(dmask, 0.0)
        for j in range(4):
            nc.gpsimd.affine_select(
                out=dmask[:, j, :],
                in_=dmask[:, j, :],
                compare_op=mybir.AluOpType.is_ge,
                fill=NEG,
                base=0,
                pattern=[[1, P]],
                channel_multiplier=-1,
            )

    # ---- weights / biases -----------------------------------------------
    def load_w(dst, ap):
        if MDT == BF16:
            nc.gpsimd.dma_start(dst, ap)          # SWDGE casts f32 -> bf16
        else:
            nc.sync.dma_start(dst, ap.bitcast(MDT))

    w_sb = {}
    for name in ("wq", "wk", "wv"):
        w_sb[name] = persist.tile([P, ND, DHH], MDT, name=f"w_{name}")
        load_w(w_sb[name], io[name].rearrange("(a p) o -> p a o", p=P))
    # wo: full-D rows, this core's DHH output columns
    wo_sb = persist.tile([P, ND, DHH], MDT)
    load_w(wo_sb, io["wo"].rearrange("(a p) o -> p a o", p=P))

    b_sb = {}
    if with_bias:
        for name in ("bq", "bk", "bv"):
            b_sb[name] = const.tile([1, DHH], MDT, name=f"b_{name}")
            load_w(b_sb[name], io[name])

    # ---- persistent activations: one tile per seq-chunk -----------------
    qT = [persist.tile([P, NH2, FD], MDT, name=f"qT{i}") for i in range(NQ)]
    kT = [persist.tile([P, NH2, FD], MDT, name=f"kT{i}") for i in range(NQ)]
    v_c = [persist.tile([P, SPC, HPC, DH], MDT, name=f"v{i}")
           for i in range(NQ)]
    ctxT = [persist.tile([P, NH2, FD], MDT, name=f"ctxT{i}")
            for i in range(NQ)]

    scale = 1.0 / float(np.sqrt(DH))
    # collective staging.  The first collective op also pays a ~50us
    # one-time comm-init that runs concurrently with early compute, so the
    # first TWO chunks share one AllGather triggered after chunk 1 -- by
    # then the init has finished in the shadow of chunk 0/1 compute.
    merge_first = NQ >= 2
    nmerge = 2 if merge_first else 1
    stage01 = dram.tile([nmerge, P, NH2, FD], MDT, name="ctx_stage01")
    gath01 = dram.tile([TP, nmerge, P, NH2, FD], MDT, name="ctx_gath01")
    ctx_stage = {i: dram.tile([P, NH2, FD], MDT, name=f"ctx_stage_{i}")
                 for i in range(nmerge, NQ)}
    ctx_gath = {i: dram.tile([TP, P, NH2, FD], MDT, name=f"ctx_gath_{i}")
                for i in range(nmerge, NQ)}
    groups = [list(range(g * TP, (g + 1) * TP)) for g in range(DP)]

    with (
        tc.tile_pool(name="xt", bufs=2) as xt_pool,
        tc.tile_pool(name="xth", bufs=1) as xth_pool,
        tc.tile_pool(name="mm_ps", bufs=1, space="PSUM") as mm_ps_pool,
        tc.tile_pool(name="sc_ps", bufs=2, space="PSUM") as sc_ps_pool,
        tc.tile_pool(name="ctx_ps", bufs=3, space="PSUM") as ctx_ps_pool,
        tc.tile_pool(name="pt", bufs=4) as pt_pool,
        tc.tile_pool(name="mload", bufs=3) as mload_pool,
        tc.tile_pool(name="small", bufs=2) as small_pool,
        tc.tile_pool(name="cg_sb", bufs=4) as cg_pool,
        tc.tile_pool(name="out_sb", bufs=3) as out_sb_pool,
    ):
        # hoist all x^T loads ahead of the stream loop: the bf16 cast-DMAs
        # run on the GPSIMD queue, which also issues the collectives -- if
        # emitted inside the loop they stall behind each AllGather.
        xt_all = {}
        if MDT == BF16:
            for sc in range(NQ):
                for tname in ("xq", "xk", "xv"):
                    xt_c = xth_pool.tile([P, ND, FD], MDT,
                                         tag=f"xt_{tname}_{sc}",
                                         name=f"xt_{tname}_{sc}")
                    nc.gpsimd.dma_start(
                        xt_c,
                        io[tname].rearrange("(a p) t -> p a t", p=P)[
                            :, :, sc * FD:(sc + 1) * FD
                        ],
                    )
                    xt_all[(tname, sc)] = xt_c

        def project_chunk(sc):
            for tname, wname, bname, dstT in (
                ("xq", "wq", "bq", qT),
                ("xk", "wk", "bk", kT),
                ("xv", "wv", "bv", None),
            ):
                if MDT == BF16:
                    xt_c = xt_all[(tname, sc)]
                else:
                    xt_c = xt_pool.tile([P, ND, FD], MDT, tag="xt",
                                        name=f"xt_{tname}_{sc}")
                    nc.sync.dma_start(
                        xt_c,
                        io[tname].rearrange("(a p) t -> p a t", p=P)[
                            :, :, sc * FD:(sc + 1) * FD
                        ].bitcast(MDT),
                    )
                if dstT is not None:
                    for mt in range(NH2):
                        qps = mm_ps_pool.tile([P, FD], F32, tag="mm",
                                              name=f"qps_{tname}_{sc}_{mt}")
                        for dt in range(ND):
                            nc.tensor.matmul(
                                qps,
                                w_sb[wname][:, dt, mt * P:(mt + 1) * P],
                                xt_c[:, dt, :],
                                start=(dt == 0),
                                stop=(not with_bias and dt == ND - 1),
                            )
                        if with_bias:
                            nc.tensor.matmul(  # + bias (ones-row augment)
                                qps,
                                b_sb[bname][0:1, mt * P:(mt + 1) * P],
                                ones[0:1, :],
                                start=False,
                                stop=True,
                            )
                        nc.vector.tensor_copy(dstT[sc][:, mt, :], qps)
                else:
                    for st in range(SPC):
                        vp = mm_ps_pool.tile([P, DHH], F32, tag="mm",
                                             name=f"vps_{sc}_{st}")
                        for dt in range(ND):
                            nc.tensor.matmul(
                                vp,
                                xt_c[:, dt, st * P:(st + 1) * P],
                                w_sb[wname][:, dt, :],
                                start=(dt == 0),
                                stop=(not with_bias and dt == ND - 1),
                            )
                        if with_bias:
                            nc.tensor.matmul(
                                vp,
                                ones[0:1, 0:P],
                                b_sb[bname][0:1, :],
                                start=False,
                                stop=True,
                            )
                        nc.vector.tensor_copy(
                            v_c[sc][:, st, :, :],
                            vp.rearrange("p (h e) -> p h e", h=HPC),
                        )

        def attend_chunk(qc):
            nkt = (qc + 1) * SPC if mask_mode == "causal" else NK
            # ctx[p]: two heads of pair p col-tiled into one bank
            # den: 4 heads' softmax denominators at partitions 0/32/64/96
            ctx = [
                ctx_ps_pool.tile([P, FD], F32, tag="ctx",
                                 name=f"ctx_{qc}_{p_}")
                for p_ in range(NH2)
            ]
            den = ctx_ps_pool.tile([P, FD], F32, tag="ctx",
                                   name=f"den_{qc}")
            # one full-bank clearing matmul per accumulator bank (zeros with
            # every has_written bit set) so the col-tiled partial-partition
            # matmuls below can all run start=False -- no ordering hazard
            # between accumulation groups sharing a bank.
            for acc in (ctx[0], ctx[1], den):
                nc.tensor.matmul(acc, zrow, orow, start=True, stop=False,
                                 skip_group_check=True)
            for kt in range(nkt):
                ksc, kti = kt // SPC, kt % SPC
                dj = kt - qc * SPC
                mt_sb = None
                if mask_mode == "generic":
                    mt_sb = mload_pool.tile([P, FD], F32, tag="ml")
                    nc.sync.dma_start(
                        mt_sb,
                        io["maskT"][kt * P:(kt + 1) * P,
                                    qc * FD:(qc + 1) * FD],
                    )
                # causal diagonal tiles: queries below 128*dj see nothing
                # of this key tile -- compute only the valid q-range and
                # mask only the [P, P] sub-tile crossing the diagonal.
                q0 = P * dj if (mask_mode == "causal" and dj > 0) else 0
                for p_ in range(NH2):
                    sp = sc_ps_pool.tile([P, 2, FD], F32, tag="sc",
                                         name=f"sc_{qc}_{kt}_{p_}")
                    for j in range(2):
                        nc.tensor.matmul(
                            sp[:, j, q0:FD],
                            kT[ksc][64 * j:64 * (j + 1), p_,
                                    kti * P:(kti + 1) * P],
                            qT[qc][64 * j:64 * (j + 1), p_, q0:FD],
                            start=True,
                            stop=True,
                        )
                    if mt_sb is not None:
                        for j in range(2):
                            nc.vector.tensor_add(sp[:, j, :], sp[:, j, :],
                                                 mt_sb)
                    elif mask_mode == "causal" and dj >= 0:
                        for j in range(2):
                            nc.vector.tensor_add(
                                sp[:, j, q0:q0 + P], sp[:, j, q0:q0 + P],
                                dmask[:, dj, 0:P],
                            )
                    pt = pt_pool.tile([P, 2, FD], MDT, tag="pt")
                    if q0 == 0:
                        # contiguous 2-bank tile: one flat free dim avoids
                        # the ACT per-row restart overhead
                        nc.scalar.activation(
                            pt.rearrange("p a b -> p (a b)"),
                            sp.rearrange("p a b -> p (a b)"),
                            AF.Exp, scale=scale,
                        )
                    else:
                        nc.scalar.activation(pt[:, :, q0:FD], sp[:, :, q0:FD],
                                             AF.Exp, scale=scale)
                    for j in range(2):
                        hj = 2 * p_ + j
                        nc.tensor.matmul(  # PV, col-tiled pair
                            ctx[p_][64 * j:64 * (j + 1), q0:FD],
                            v_c[ksc][:, kti, hj, :],
                            pt[:, j, q0:FD],
                            start=False,
                            stop=(kt == nkt - 1),
                            skip_group_check=True,
                        )
                        nc.tensor.matmul(  # denominator, col-tiled 4-way
                            den[32 * hj:32 * hj + 1, q0:FD],
                            den_ones,
                            pt[:, j, q0:FD],
                            start=False,
                            stop=(kt == nkt - 1),
                            tile_position=(0, 32 * hj),
                            skip_group_check=True,
                        )
            # normalize: ctxT = ctx * (1/den), broadcast along partitions
            # via a tiny PE matmul (keeps GPSIMD free for collectives).
            # One partition-parallel reciprocal covers all 4 heads (rows
            # 0/32/64/96; the other rows are zeros from the bank clear and
            # their 1/0=inf results are never read).
            nc.vector.reciprocal(recip_f, den[0:97, :])
            # bc tiles live in the score pool (free after the last exp) so
            # the mm pool's FIFO stays clear for the next chunk's projections
            bc_t = sc_ps_pool.tile([P, 2, FD], F32, tag="sc",
                                   name=f"bc_{qc}")
            for p_ in range(NH2):
                recip = recip_sb[p_]
                for j in range(2):
                    hj = 2 * p_ + j
                    nc.vector.tensor_copy(
                        recip[32 * j:32 * j + 1, :],
                        recip_f[32 * hj:32 * hj + 1, :],
                    )
                nc.tensor.matmul(
                    bc_t[:, p_, :],
                    sel,
                    recip,
                    start=True,
                    stop=True,
                )
                bc_sb = small_pool.tile([P, FD], F32, tag="bc_sb",
                                        name=f"bc_sb_{qc}_{p_}")
                nc.vector.tensor_copy(bc_sb, bc_t[:, p_, :])
                nc.vector.tensor_mul(ctxT[qc][:, p_, :], ctx[p_], bc_sb)

        def exchange_chunk(qc):
            if qc < nmerge:
                # merged exchange for chunks [0, nmerge): one AllGather
                for i in range(nmerge):
                    nc.gpsimd.dma_start(stage01[i], ctxT[i])
                nc.gpsimd.collective_compute(
                    "AllGather",
                    mybir.AluOpType.bypass,
                    replica_groups=groups,
                    ins=[stage01.opt()],
                    outs=[gath01.opt()],
                )
            else:
                nc.gpsimd.dma_start(ctx_stage[qc], ctxT[qc])
                nc.gpsimd.collective_compute(
                    "AllGather",
                    mybir.AluOpType.bypass,
                    replica_groups=groups,
                    ins=[ctx_stage[qc].opt()],
                    outs=[ctx_gath[qc].opt()],
                )

        cg_all = {}

        def load_cg(qc):
            # gathered ctx^T -> SBUF, on the gpsimd queue: emitted right
            # after the NEXT chunk's AG trigger, its wait (this chunk's
            # AG done) always releases before that trigger's own wait --
            # unlike the sync queue, where it would queue behind later
            # chunks' staging DMAs and strand the output projections.
            if qc < nmerge:
                src = gath01[:, qc].rearrange("r p h q -> p r h q")
            else:
                src = ctx_gath[qc].rearrange("r p h q -> p r h q")
            cg = cg_pool.tile([P, TP, NH2, FD], MDT, tag="cg",
                              name=f"cg_{qc}")
            nc.gpsimd.dma_start(cg, src)
            cg_all[qc] = cg

        def project_out_chunk(qc):
            # every core projects ALL queries of this chunk against its
            # own 256 Wo columns (rank-dependence is in the Wo input).
            cg = cg_all[qc]
            for st in range(SPC):
                op = mm_ps_pool.tile([P, DHH], F32, tag="mm",
                                     name=f"op_{qc}_{st}")
                for r in range(TP):
                    for hp in range(NH2):
                        nc.tensor.matmul(
                            op,
                            cg[:, r, hp, st * P:(st + 1) * P],
                            wo_sb[:, 2 * r + hp, :],
                            start=(r == 0 and hp == 0),
                            stop=(r == TP - 1 and hp == NH2 - 1),
                        )
                ob = out_sb_pool.tile([P, DHH], F32, tag="ob")
                nc.vector.tensor_copy(ob, op)
                nc.sync.dma_start(
                    io["out"][qc, st * P:(st + 1) * P, :], ob
                )

        # stream: chunk qc's attention needs only K/V chunks <= qc (causal),
        # so interleave projection and attention per chunk.  The output
        # projection of chunk qc is emitted two chunks later: its PSUM
        # tiles wait on the AllGather, and emitting them late keeps that
        # wait from serializing the (FIFO) psum pool rotation.
        def emit_exchange(sc):
            if merge_first:
                if sc == 1:
                    exchange_chunk(0)
                    load_cg(0)
                    load_cg(1)
                elif sc >= 2:
                    exchange_chunk(sc)
                    load_cg(sc)
            else:
                exchange_chunk(sc)
                load_cg(sc)

        if mask_mode == "causal":
            for sc in range(NQ):
                project_chunk(sc)
                attend_chunk(sc)
                emit_exchange(sc)
        else:
            for sc in range(NQ):
                project_chunk(sc)
            for qc in range(NQ):
                attend_chunk(qc)
                emit_exchange(qc)
        # all output projections AFTER the stream: their PSUM tiles queue
        # behind the last projections in the (FIFO) mm pool, so they fill
        # the last chunk's exp-bound PE gaps instead of blocking the stream
        # on AllGather completion.
        for qc in range(NQ):
            project_out_chunk(qc)


def build(mask_mode="causal", s=S, mm_dtype="f32r", with_bias=True):
    """Build the SPMD Bass module for one core."""
    assert mask_mode in ("causal", "zeros", "generic")
    assert mm_dtype in ("f32r", "bf16", "f32")
    assert s % FD == 0
    nc = bacc.Bacc(
        "TRN2", target_bir_lowering=False, debug=False, num_devices=N_CORES
    )
    io = {}
    for name in ("xq", "xk", "xv"):
        # host passes x^T: [D, s]
        io[name] = nc.dram_tensor(name, [D, s], F32, kind="ExternalInput").ap()
    for name in ("wq", "wk", "wv"):
        io[name] = nc.dram_tensor(name, [D, DHH], F32, kind="ExternalInput").ap()
    # wo: full rows, this core's DHH columns
    io["wo"] = nc.dram_tensor("wo", [D, DHH], F32, kind="ExternalInput").ap()
    for name in ("bq", "bk", "bv"):
        io[name] = nc.dram_tensor(name, [1, DHH], F32, kind="ExternalInput").ap()
    if mask_mode == "generic":
        io["maskT"] = nc.dram_tensor(
            "maskT", [s, s], F32, kind="ExternalInput"
        ).ap()
    # output: all queries, this core's DHH output columns
    io["out"] = nc.dram_tensor(
        "out", [s // FD, FD, DHH], F32, kind="ExternalOutput"
    ).ap()

    with tile.TileContext(nc) as tc:
        _emit(tc, io, mask_mode, s, mm_dtype, with_bias)
    nc.compile()
    return nc


def detect_mask_mode(mask, s=S):
    m = np.asarray(mask).reshape(s, s)
    if not np.any(m):
        return "zeros"
    causal = np.where(
        np.tril(np.ones((s, s), dtype=bool)), 0.0, np.float32(NEG)
    ).astype(np.float32)
    if np.array_equal(m, causal):
        return "causal"
    return "generic"


def make_in_maps(q, k, v, mask, Wq, bq, Wk, bk, Wv, bv, Wo, bo, mask_mode,
                 s=S):
    c32 = lambda a: np.ascontiguousarray(a, dtype=np.float32)
    # one host-side transpose per (batch, tensor), shared by the TP group
    xT = [[c32(np.asarray(t)[g].T) for t in (q, k, v)] for g in range(DP)]
    in_maps = []
    for c in range(N_CORES):
        g, r = c // TP, c % TP
        sl = slice(r * DHH, (r + 1) * DHH)
        m = {
            "xq": xT[g][0], "xk": xT[g][1], "xv": xT[g][2],
            "wq": c32(Wq[:, sl]), "wk": c32(Wk[:, sl]), "wv": c32(Wv[:, sl]),
            "wo": c32(Wo[:, sl]),
            "bq": c32(bq[sl]).reshape(1, DHH),
            "bk": c32(bk[sl]).reshape(1, DHH),
            "bv": c32(bv[sl]).reshape(1, DHH),
        }
        if mask_mode == "generic":
            # pre-scaled by sqrt(DH) so exp((s + m*8)/8) == exp(s/8 + m)
            m["maskT"] = c32(
                np.asarray(mask).reshape(s, s).T * np.float32(DH) ** 0.5
            )
        in_maps.append(m)
    return in_maps


def assemble(results, bo, s=S):
    out = np.empty((B, s, D), np.float32)
    for c in range(N_CORES):
        g, r = c // TP, c % TP
        piece = np.asarray(results[c]["out"]).reshape(s, DHH)
        out[g, :, r * DHH:(r + 1) * DHH] = piece
    out += np.asarray(bo, dtype=np.float32)[None, None, :]
    return out


_cache = {}
MM_DTYPE = "bf16"


def kernel(q, k, v, mask, Wq, bq, Wk, bk, Wv, bv, Wo, bo):
    mask_mode = detect_mask_mode(mask)
    with_bias = any(np.any(np.asarray(b)) for b in (bq, bk, bv))
    key = (mask_mode, with_bias)
    if key not in _cache:
        _cache[key] = build(mask_mode=mask_mode, mm_dtype=MM_DTYPE,
                            with_bias=with_bias)
    nc = _cache[key]
    in_maps = make_in_maps(
        q, k, v, mask, Wq, bq, Wk, bk, Wv, bv, Wo, bo, mask_mode
    )
    res = run_bass_kernel_spmd(nc, in_maps, list(range(N_CORES)))
    return assemble(res.results, bo)
